# revision 7
# baseline (speedup 1.0000x reference)
"""Trainium2 Bass kernel for GQA attention (B=2, S=2048, HID=2048, H=16, HKV=4, RoPE, causal).

Sharding: TP=4 over GQA groups (4 Q heads + 1 KV head per core) x DP=2 over batch.
Core i -> (batch = i // 4, group = i % 4). Each core computes a partial output
x @ Wo_shard for its head group; host sums the 4 partials per batch.

Per-core pipeline (all matmul operands fp16, fp32 PSUM accumulation):
  Phase P: qkv projections (xT chunks stationary, packed [Wq|Wk|Wv] moving),
           RoPE on DVE in [s, d] layout (free-axis half swap, host-signed sin),
           PE-transpose q/k head blocks -> qT/kT [d, s] fp16; v stays [s, d].
  Phase A: per 512-row query strip x head: scores = qT.T @ kT strips (N=512),
           causal mask via 4 precomputed diag-box masks (DVE add),
           exp on ACT (scale=1/sqrt(d), bias=-4) with fused row-sum accum,
           normalize P (tensor_scalar by reciprocal row-sum) -> fp16,
           PE-transpose P blocks -> PT [sk, sq], PV with v chunks stationary,
           then O-projection using PV's [d, sq] output directly as stationary.
"""
import sys
sys.path.insert(0, "/opt/trn_rl_repo")
import math
import numpy as np
import concourse.bass as bass
import concourse.mybir as mybir
import concourse.tile as tile
from concourse import bacc
from concourse.bass_utils import run_bass_kernel_spmd
from concourse.masks import make_identity

F16 = mybir.dt.float16
F32 = mybir.dt.float32
AF = mybir.ActivationFunctionType
ALU = mybir.AluOpType

NH = 4          # q heads per core
D = 128         # head dim
MASK_VAL = -1e9
EXP_BIAS = -4.0


def build(S=2048, HID=2048):
    SC = S // 128        # seq chunks
    NT = S // 512        # 512-wide query strips
    HC = HID // 128      # hidden (contraction) chunks
    QW = NH * D          # 512: q width per core
    scale = 1.0 / math.sqrt(D)

    nc = bacc.Bacc(None, target_bir_lowering=False, debug=False)
    with tile.TileContext(nc) as tc:
        with tc.tile_pool(name="dram", bufs=1, space="DRAM") as dram:
            xt_d = dram.tile([128, HC * S], F16, kind="ExternalInput", name="xt", uniquify=False)
            wqkv_d = dram.tile([128, HC * (QW + 256)], F16, kind="ExternalInput", name="wqkv", uniquify=False)
            cos_d = dram.tile([128, SC * QW], F16, kind="ExternalInput", name="cos4", uniquify=False)
            sin_d = dram.tile([128, SC * QW], F16, kind="ExternalInput", name="sin4", uniquify=False)
            wo_d = dram.tile([128, NH * HID], F16, kind="ExternalInput", name="wo", uniquify=False)
            out_d = dram.tile([S, HID], F32, kind="ExternalOutput", name="out", uniquify=False)

            # ---- persistent sbuf ----
            with tc.tile_pool(name="keep", bufs=1) as keep:
                qT_sb = keep.tile([128, NH * S], F16)   # [d, h*S + sq]
                kT_sb = keep.tile([128, S], F16)        # [d, sk]
                v_sb = keep.tile([128, S], F16)         # [sk%128, chunk*128 + d]
                ident = keep.tile([128, 128], F16)
                make_identity(nc, ident[:])
                ebias = keep.tile([128, 1], F32)
                nc.gpsimd.memset(ebias[:], EXP_BIAS)
                masks = keep.tile([128, 4 * 512], F32)  # 4 diag-box masks [128,512]
                nc.gpsimd.memset(masks[:], 0.0)
                for m in range(4):
                    nc.gpsimd.affine_select(
                        out=masks[:, m * 512:(m + 1) * 512],
                        in_=masks[:, m * 512:(m + 1) * 512],
                        compare_op=ALU.is_ge,
                        fill=MASK_VAL,
                        base=m * 128,
                        pattern=[[-1, 512]],
                        channel_multiplier=1,
                    )

                # ---- phase P: projections + rope + transposes ----
                with tc.tile_pool(name="pp", bufs=1) as pp, \
                     tc.tile_pool(name="psp", bufs=2, space="PSUM") as psp:
                    xt_sb = pp.tile([128, HC * S], F16)
                    wqkv_sb = pp.tile([128, HC * (QW + 256)], F16)
                    cos_sb = pp.tile([128, SC * QW], F16)
                    sin_sb = pp.tile([128, SC * QW], F16)
                    nc.sync.dma_start(out=xt_sb[:], in_=xt_d[:])
                    nc.sync.dma_start(out=wqkv_sb[:], in_=wqkv_d[:])
                    nc.sync.dma_start(out=cos_sb[:], in_=cos_d[:])
                    nc.sync.dma_start(out=sin_sb[:], in_=sin_d[:])

                    for c in range(SC):
                        q_ps = psp.tile([128, QW], F32, tag="qps")
                        kv_ps = psp.tile([128, 256], F32, tag="kvps")
                        for hh in range(HC):
                            xk = xt_sb[:, hh * S + c * 128: hh * S + (c + 1) * 128]
                            nc.tensor.matmul(q_ps[:], xk, wqkv_sb[:, hh * (QW + 256): hh * (QW + 256) + QW],
                                             start=(hh == 0), stop=(hh == HC - 1))
                            nc.tensor.matmul(kv_ps[:], xk, wqkv_sb[:, hh * (QW + 256) + QW: (hh + 1) * (QW + 256)],
                                             start=(hh == 0), stop=(hh == HC - 1))
                        # --- RoPE on q (4 heads batched) ---
                        q4 = q_ps[:].rearrange("p (h d) -> p h d", h=NH)
                        cos4v = cos_sb[:, c * QW:(c + 1) * QW].rearrange("p (h d) -> p h d", h=NH)
                        sin4v = sin_sb[:, c * QW:(c + 1) * QW].rearrange("p (h d) -> p h d", h=NH)
                        rot = pp.tile([128, QW], F32, tag="rot")
                        rot4 = rot[:].rearrange("p (h d) -> p h d", h=NH)
                        nc.vector.tensor_mul(rot4[:, :, 0:64], q4[:, :, 64:128], sin4v[:, :, 0:64])
                        nc.vector.tensor_mul(rot4[:, :, 64:128], q4[:, :, 0:64], sin4v[:, :, 64:128])
                        qc = pp.tile([128, QW], F32, tag="qc")
                        nc.vector.tensor_mul(qc[:], q_ps[:], cos_sb[:, c * QW:(c + 1) * QW])
                        q16 = pp.tile([128, QW], F16, tag="q16")
                        nc.vector.tensor_add(q16[:], qc[:], rot[:])
                        # --- RoPE on k (head 0 slices of cos/sin) ---
                        k1 = kv_ps[:, 0:128]
                        cos1 = cos_sb[:, c * QW: c * QW + 128]
                        sin1 = sin_sb[:, c * QW: c * QW + 128]
                        krot = pp.tile([128, 128], F32, tag="krot")
                        nc.vector.tensor_mul(krot[:, 0:64], k1[:, 64:128], sin1[:, 0:64])
                        nc.vector.tensor_mul(krot[:, 64:128], k1[:, 0:64], sin1[:, 64:128])
                        kc = pp.tile([128, 128], F32, tag="kc")
                        nc.vector.tensor_mul(kc[:], k1, cos1)
                        k16 = pp.tile([128, 128], F16, tag="k16")
                        nc.vector.tensor_add(k16[:], kc[:], krot[:])
                        # --- v to persistent [s, d] fp16 ---
                        nc.vector.tensor_copy(v_sb[:, c * 128:(c + 1) * 128], kv_ps[:, 128:256])
                        # --- transpose q heads + k into qT/kT ---
                        tr_ps = psp.tile([128, 640], F16, tag="trps")
                        for h in range(NH):
                            nc.tensor.transpose(tr_ps[:, h * 128:(h + 1) * 128], q16[:, h * 128:(h + 1) * 128], ident[:])
                        nc.tensor.transpose(tr_ps[:, 512:640], k16[:], ident[:])
                        qT_view = qT_sb[:].rearrange("p (h s) -> p h s", h=NH)[:, :, c * 128:(c + 1) * 128]
                        nc.vector.tensor_copy(qT_view, tr_ps[:, 0:512].rearrange("p (h s) -> p h s", h=NH))
                        nc.vector.tensor_copy(kT_sb[:, c * 128:(c + 1) * 128], tr_ps[:, 512:640])

                # ---- phase A: attention + output projection ----
                with tc.tile_pool(name="pa", bufs=1) as pa, \
                     tc.tile_pool(name="psa", bufs=2, space="PSUM") as psa:
                    wo_sb = pa.tile([128, NH * HID], F16)
                    nc.sync.dma_start(out=wo_sb[:], in_=wo_d[:])
                    for t in range(NT):
                        nk = 4 * t + 4            # sk chunks in play for this strip
                        width = nk * 128          # score row width
                        attnT = pa.tile([128, NH * 512], F16, tag="attnT")
                        for h in range(NH):
                            pt_sb = pa.tile([128, nk * 512], F16, tag="pt")
                            prow = pa.tile([128, S], F16, tag="prow")
                            rs = pa.tile([128, 4], F32, tag="rs")
                            for ci in range(4):
                                c = 4 * t + ci
                                qTc = qT_sb[:, h * S + c * 128: h * S + (c + 1) * 128]
                                for j in range(t + 1):
                                    sc = psa.tile([128, 512], F32, tag="sc")
                                    nc.tensor.matmul(sc[:], qTc, kT_sb[:, j * 512:(j + 1) * 512],
                                                     start=True, stop=True)
                                    if j == t:
                                        nc.vector.tensor_add(sc[:], sc[:], masks[:, ci * 512:(ci + 1) * 512])
                                    nc.scalar.activation(prow[:, j * 512:(j + 1) * 512], sc[:], AF.Exp,
                                                         scale=scale, bias=ebias[:],
                                                         accum_out=rs[:, j:j + 1])
                                rst = pa.tile([128, 1], F32, tag="rst")
                                if t == 0:
                                    nc.vector.tensor_copy(rst[:], rs[:, 0:1])
                                elif t == 1:
                                    nc.vector.tensor_add(rst[:], rs[:, 0:1], rs[:, 1:2])
                                else:
                                    nc.vector.tensor_add(rst[:], rs[:, 0:1], rs[:, 1:2])
                                    for j in range(2, t + 1):
                                        nc.vector.tensor_add(rst[:], rst[:], rs[:, j:j + 1])
                                rcp = pa.tile([128, 1], F32, tag="rcp")
                                nc.vector.reciprocal(rcp[:], rst[:])
                                pn = pa.tile([128, S], F16, tag="pn")
                                nc.vector.tensor_scalar_mul(pn[:, 0:width], prow[:, 0:width], rcp[:])
                                # transpose P blocks of this row into PT strip layout
                                for k0 in range(0, nk, 4):
                                    kb = min(4, nk - k0)
                                    tp = psa.tile([128, 512], F16, tag="tp")
                                    for kk in range(kb):
                                        nc.tensor.transpose(tp[:, kk * 128:(kk + 1) * 128],
                                                            pn[:, (k0 + kk) * 128:(k0 + kk + 1) * 128], ident[:])
                                    ptv = pt_sb[:].rearrange("p (k s) -> p k s", s=512)[:, k0:k0 + kb, ci * 128:(ci + 1) * 128]
                                    nc.vector.tensor_copy(ptv, tp[:, 0:kb * 128].rearrange("p (k s) -> p k s", s=128))
                            # PV for this (h, t)
                            pv = psa.tile([128, 512], F32, tag="pv")
                            for k in range(nk):
                                nc.tensor.matmul(pv[:], v_sb[:, k * 128:(k + 1) * 128],
                                                 pt_sb[:, k * 512:(k + 1) * 512],
                                                 start=(k == 0), stop=(k == nk - 1))
                            nc.vector.tensor_copy(attnT[:, h * 512:(h + 1) * 512], pv[:])
                        # O projection for the strip
                        for ci in range(4):
                            c = 4 * t + ci
                            osb = pa.tile([128, HID], F32, tag="osb")
                            for n in range(HID // 512):
                                op = psa.tile([128, 512], F32, tag="op")
                                for h in range(NH):
                                    nc.tensor.matmul(op[:], attnT[:, h * 512 + ci * 128: h * 512 + (ci + 1) * 128],
                                                     wo_sb[:, h * HID + n * 512: h * HID + (n + 1) * 512],
                                                     start=(h == 0), stop=(h == NH - 1))
                                nc.vector.tensor_copy(osb[:, n * 512:(n + 1) * 512], op[:])
                            nc.sync.dma_start(out=out_d[c * 128:(c + 1) * 128, :], in_=osb[:])
    nc.compile()
    return nc


def _chunk_major(a, rows=128):
    """[R, C] -> [128, (R//128)*C] with row-chunk-major free layout."""
    r, c = a.shape
    return np.ascontiguousarray(a.reshape(r // rows, rows, c).transpose(1, 0, 2).reshape(rows, (r // rows) * c))


def make_in_map(x_b, cos, sin, wq_g, wk_g, wv_g, wo_g, S, HID):
    SC = S // 128
    xt = _chunk_major(np.ascontiguousarray(x_b.T)).astype(np.float16)
    wqkv = _chunk_major(np.concatenate([wq_g, wk_g, wv_g], axis=1)).astype(np.float16)
    cosr = cos[:S].reshape(SC, 128, D)
    cos4 = np.repeat(cosr[:, :, None, :], NH, axis=2).transpose(1, 0, 2, 3).reshape(128, SC * NH * D)
    sing = np.concatenate([-sin[:S, :64], sin[:S, 64:]], axis=1).reshape(SC, 128, D)
    sin4 = np.repeat(sing[:, :, None, :], NH, axis=2).transpose(1, 0, 2, 3).reshape(128, SC * NH * D)
    wo = _chunk_major(wo_g).astype(np.float16)
    return {
        "xt": xt,
        "wqkv": wqkv,
        "cos4": np.ascontiguousarray(cos4).astype(np.float16),
        "sin4": np.ascontiguousarray(sin4).astype(np.float16),
        "wo": wo,
    }


_NC_CACHE = {}


def _get_nc(S, HID):
    key = (S, HID)
    if key not in _NC_CACHE:
        _NC_CACHE[key] = build(S, HID)
    return _NC_CACHE[key]


def kernel(x, cos, sin, Wq, Wk, Wv, Wo):
    x = np.asarray(x, dtype=np.float32)
    cos = np.asarray(cos, dtype=np.float32)
    sin = np.asarray(sin, dtype=np.float32)
    Wq = np.asarray(Wq, dtype=np.float32)
    Wk = np.asarray(Wk, dtype=np.float32)
    Wv = np.asarray(Wv, dtype=np.float32)
    Wo = np.asarray(Wo, dtype=np.float32)
    B, S, HID = x.shape

    in_maps = []
    for i in range(8):
        b, g = i // 4, i % 4
        in_maps.append(make_in_map(
            x[b], cos, sin,
            Wq[:, g * NH * D:(g + 1) * NH * D],
            Wk[:, g * D:(g + 1) * D],
            Wv[:, g * D:(g + 1) * D],
            Wo[g * NH * D:(g + 1) * NH * D, :],
            S, HID))

    nc = _get_nc(S, HID)
    last_err = None
    for _attempt in range(3):
        try:
            res = run_bass_kernel_spmd(nc, in_maps, core_ids=list(range(8)), trace=False)
            break
        except Exception as e:  # flaky NRT_EXEC_UNIT_UNRECOVERABLE seen on first runs
            last_err = e
    else:
        raise last_err
    out = np.zeros((B, S, HID), dtype=np.float32)
    for i in range(8):
        b = i // 4
        out[b] += res.results[i]["out"]
    return out


# revision 15
# speedup vs baseline: 6.8079x; 6.8079x over previous
"""Trainium2 Bass kernel for GQA attention (B=2, S=2048, HID=2048, H=16, HKV=4, RoPE, causal).

Sharding: TP=4 over GQA groups (4 Q heads + 1 KV head per core) x DP=2 over batch.
Core i -> (batch = i // 4, group = i % 4). Each core computes a partial output
x @ Wo_shard for its head group; host sums the 4 partials per batch.

Per-core pipeline (all matmul operands fp16, fp32 PSUM accumulation):
  Phase P: qkv projections (xT chunks stationary, packed [Wq|Wk|Wv] moving),
           RoPE on DVE in [s, d] layout (free-axis half swap, host-signed sin),
           PE-transpose q/k head blocks -> qT/kT [d, s] fp16; v stays [s, d].
  Phase A: per 512-row query strip x head: scores = qT.T @ kT strips (N=512),
           causal mask via 4 precomputed diag-box masks (DVE add),
           exp on ACT (scale=1/sqrt(d), bias=-4) with fused row-sum accum,
           normalize P (tensor_scalar by reciprocal row-sum) -> fp16,
           PE-transpose P blocks -> PT [sk, sq], PV with v chunks stationary,
           then O-projection using PV's [d, sq] output directly as stationary.
"""
import sys
sys.path.insert(0, "/opt/trn_rl_repo")
import math
import numpy as np
import concourse.bass as bass
import concourse.mybir as mybir
import concourse.tile as tile
from concourse import bacc
from concourse.bass_utils import run_bass_kernel_spmd
from concourse.masks import make_identity

F16 = mybir.dt.float16
F32 = mybir.dt.float32
AF = mybir.ActivationFunctionType
ALU = mybir.AluOpType

NH = 4          # q heads per core
D = 128         # head dim
MASK_VAL = -1e9
EXP_BIAS = -4.0


DEFAULT_BUFS = dict(sc=2, tp=2, pv=2, op=2, prow=2, pn=2, pt=2, attnT=2,
                    qps=2, kvps=2, trps=2, osb=2)


def build(S=2048, HID=2048, repeat=1, bufs=None):
    bz = dict(DEFAULT_BUFS)
    if bufs:
        bz.update(bufs)
    SC = S // 128        # seq chunks
    NT = S // 512        # 512-wide query strips
    HC = HID // 128      # hidden (contraction) chunks
    QW = NH * D          # 512: q width per core
    scale = 1.0 / math.sqrt(D)

    nc = bacc.Bacc(None, target_bir_lowering=False, debug=False)
    with tile.TileContext(nc) as tc:
        with tc.tile_pool(name="dram", bufs=1, space="DRAM") as dram:
            xt_d = dram.tile([128, HC * S], F16, kind="ExternalInput", name="xt", uniquify=False)
            wqkv_d = dram.tile([128, HC * (QW + 256)], F16, kind="ExternalInput", name="wqkv", uniquify=False)
            cos_d = dram.tile([128, SC * QW], F16, kind="ExternalInput", name="cos4", uniquify=False)
            sin_d = dram.tile([128, SC * QW], F16, kind="ExternalInput", name="sin4", uniquify=False)
            wo_d = dram.tile([128, NH * HID], F16, kind="ExternalInput", name="wo", uniquify=False)
            out_d = dram.tile([S, HID], F32, kind="ExternalOutput", name="out", uniquify=False)

            # ---- persistent sbuf ----
            with tc.tile_pool(name="keep", bufs=1) as keep:
                qT_sb = keep.tile([128, NH * S], F16)   # [d, h*S + sq]
                kT_sb = keep.tile([128, S], F16)        # [d, sk]
                v_sb = keep.tile([128, S], F16)         # [sk%128, chunk*128 + d]
                ident = keep.tile([128, 128], F16)
                make_identity(nc, ident[:])
                ebias = keep.tile([128, 1], F32)
                nc.gpsimd.memset(ebias[:], EXP_BIAS)
                # triangular causal mask for the diagonal 128x128 block:
                # visible (0) iff key_pos <= query_pos, else MASK_VAL
                cmask = keep.tile([128, 128], F32)
                nc.gpsimd.memset(cmask[:], 0.0)
                nc.gpsimd.affine_select(
                    out=cmask[:], in_=cmask[:], compare_op=ALU.is_ge,
                    fill=MASK_VAL, base=0, pattern=[[-1, 128]], channel_multiplier=1,
                )

                # optional whole-body repeat loop (for HW timing builds)
                from contextlib import ExitStack
                _rep = ExitStack()
                if repeat > 1:
                    _rep.enter_context(tc.For_i(0, repeat, 1))

                # ---- phase P: projections + rope + transposes ----
                with tc.tile_pool(name="pp", bufs=1) as pp, \
                     tc.tile_pool(name="psp", bufs=2, space="PSUM") as psp:
                    xt_sb = pp.tile([128, HC * S], F16)
                    wqkv_sb = pp.tile([128, HC * (QW + 256)], F16)
                    cos_sb = pp.tile([128, SC * QW], F16)
                    sin_sb = pp.tile([128, SC * QW], F16)
                    nc.sync.dma_start(out=xt_sb[:], in_=xt_d[:])
                    nc.sync.dma_start(out=wqkv_sb[:], in_=wqkv_d[:])
                    nc.sync.dma_start(out=cos_sb[:], in_=cos_d[:])
                    nc.sync.dma_start(out=sin_sb[:], in_=sin_d[:])

                    for c in range(SC):
                        q_ps = psp.tile([128, QW], F32, tag="qps", bufs=bz["qps"])
                        kv_ps = psp.tile([128, 256], F32, tag="kvps", bufs=bz["kvps"])
                        for hh in range(HC):
                            xk = xt_sb[:, hh * S + c * 128: hh * S + (c + 1) * 128]
                            nc.tensor.matmul(q_ps[:], xk, wqkv_sb[:, hh * (QW + 256): hh * (QW + 256) + QW],
                                             start=(hh == 0), stop=(hh == HC - 1))
                            nc.tensor.matmul(kv_ps[:], xk, wqkv_sb[:, hh * (QW + 256) + QW: (hh + 1) * (QW + 256)],
                                             start=(hh == 0), stop=(hh == HC - 1))
                        # --- RoPE on q (4 heads batched) ---
                        q4 = q_ps[:].rearrange("p (h d) -> p h d", h=NH)
                        cos4v = cos_sb[:, c * QW:(c + 1) * QW].rearrange("p (h d) -> p h d", h=NH)
                        sin4v = sin_sb[:, c * QW:(c + 1) * QW].rearrange("p (h d) -> p h d", h=NH)
                        rot = pp.tile([128, QW], F32, tag="rot")
                        rot4 = rot[:].rearrange("p (h d) -> p h d", h=NH)
                        nc.vector.tensor_mul(rot4[:, :, 0:64], q4[:, :, 64:128], sin4v[:, :, 0:64])
                        nc.vector.tensor_mul(rot4[:, :, 64:128], q4[:, :, 0:64], sin4v[:, :, 64:128])
                        qc = pp.tile([128, QW], F32, tag="qc")
                        nc.vector.tensor_mul(qc[:], q_ps[:], cos_sb[:, c * QW:(c + 1) * QW])
                        q16 = pp.tile([128, QW], F16, tag="q16")
                        nc.vector.tensor_add(q16[:], qc[:], rot[:])
                        # --- RoPE on k (head 0 slices of cos/sin) ---
                        k1 = kv_ps[:, 0:128]
                        cos1 = cos_sb[:, c * QW: c * QW + 128]
                        sin1 = sin_sb[:, c * QW: c * QW + 128]
                        krot = pp.tile([128, 128], F32, tag="krot")
                        nc.vector.tensor_mul(krot[:, 0:64], k1[:, 64:128], sin1[:, 0:64])
                        nc.vector.tensor_mul(krot[:, 64:128], k1[:, 0:64], sin1[:, 64:128])
                        kc = pp.tile([128, 128], F32, tag="kc")
                        nc.vector.tensor_mul(kc[:], k1, cos1)
                        k16 = pp.tile([128, 128], F16, tag="k16")
                        nc.vector.tensor_add(k16[:], kc[:], krot[:])
                        # --- v to persistent [s, d] fp16 ---
                        nc.vector.tensor_copy(v_sb[:, c * 128:(c + 1) * 128], kv_ps[:, 128:256])
                        # --- transpose q heads + k into qT/kT ---
                        tr_ps = psp.tile([128, 640], F16, tag="trps", bufs=bz["trps"])
                        for h in range(NH):
                            nc.tensor.transpose(tr_ps[:, h * 128:(h + 1) * 128], q16[:, h * 128:(h + 1) * 128], ident[:])
                        nc.tensor.transpose(tr_ps[:, 512:640], k16[:], ident[:])
                        qT_view = qT_sb[:].rearrange("p (h s) -> p h s", h=NH)[:, :, c * 128:(c + 1) * 128]
                        nc.vector.tensor_copy(qT_view, tr_ps[:, 0:512].rearrange("p (h s) -> p h s", h=NH))
                        nc.vector.tensor_copy(kT_sb[:, c * 128:(c + 1) * 128], tr_ps[:, 512:640])

                # ---- phase A: attention + output projection ----
                with tc.tile_pool(name="pa", bufs=1) as pa, \
                     tc.tile_pool(name="psa", bufs=2, space="PSUM") as psa:
                    wo_sb = pa.tile([128, NH * HID], F16)
                    nc.sync.dma_start(out=wo_sb[:], in_=wo_d[:])
                    for t in range(NT):
                        nk = 4 * t + 4            # sk chunks in play for this strip
                        attnT = pa.tile([128, NH * 512], F16, tag="attnT", bufs=bz["attnT"])
                        for h in range(NH):
                            pt_sb = pa.tile([128, nk * 512], F16, tag="pt", bufs=bz["pt"])
                            prow = pa.tile([128, S], F16, tag="prow", bufs=bz["prow"])
                            rs = pa.tile([128, 4], F32, tag="rs")
                            for ci in range(4):
                                c = 4 * t + ci
                                nvis = (c + 1) * 128   # visible key width for this row block
                                qTc = qT_sb[:, h * S + c * 128: h * S + (c + 1) * 128]
                                for j in range(t + 1):
                                    w = 512 if j < t else (ci + 1) * 128
                                    sc = psa.tile([128, 512], F32, tag="sc", bufs=bz["sc"])
                                    nc.tensor.matmul(sc[:, 0:w], qTc, kT_sb[:, j * 512: j * 512 + w],
                                                     start=True, stop=True)
                                    if j == t:
                                        nc.vector.tensor_add(sc[:, ci * 128:(ci + 1) * 128],
                                                             sc[:, ci * 128:(ci + 1) * 128], cmask[:])
                                    nc.scalar.activation(prow[:, j * 512: j * 512 + w], sc[:, 0:w], AF.Exp,
                                                         scale=scale, bias=ebias[:],
                                                         accum_out=rs[:, j:j + 1])
                                rst = pa.tile([128, 1], F32, tag="rst")
                                if t == 0:
                                    nc.vector.tensor_copy(rst[:], rs[:, 0:1])
                                elif t == 1:
                                    nc.vector.tensor_add(rst[:], rs[:, 0:1], rs[:, 1:2])
                                else:
                                    nc.vector.tensor_add(rst[:], rs[:, 0:1], rs[:, 1:2])
                                    for j in range(2, t + 1):
                                        nc.vector.tensor_add(rst[:], rst[:], rs[:, j:j + 1])
                                rcp = pa.tile([128, 1], F32, tag="rcp")
                                nc.vector.reciprocal(rcp[:], rst[:])
                                pn = pa.tile([128, S], F16, tag="pn", bufs=bz["pn"])
                                nc.vector.tensor_scalar_mul(pn[:, 0:nvis], prow[:, 0:nvis], rcp[:])
                                # transpose the c+1 visible P blocks into PT strip layout
                                for k0 in range(0, c + 1, 4):
                                    kb = min(4, c + 1 - k0)
                                    tp = psa.tile([128, 512], F16, tag="tp", bufs=bz["tp"])
                                    for kk in range(kb):
                                        nc.tensor.transpose(tp[:, kk * 128:(kk + 1) * 128],
                                                            pn[:, (k0 + kk) * 128:(k0 + kk + 1) * 128], ident[:])
                                    ptv = pt_sb[:].rearrange("p (k s) -> p k s", s=512)[:, k0:k0 + kb, ci * 128:(ci + 1) * 128]
                                    nc.vector.tensor_copy(ptv, tp[:, 0:kb * 128].rearrange("p (k s) -> p k s", s=128))
                                # blocks k > c are all-zero P: memset their PT slots (Pool engine)
                                if c + 1 < nk:
                                    zv = pt_sb[:].rearrange("p (k s) -> p k s", s=512)[:, c + 1:nk, ci * 128:(ci + 1) * 128]
                                    nc.gpsimd.memset(zv, 0.0)
                            # PV for this (h, t)
                            pv = psa.tile([128, 512], F32, tag="pv", bufs=bz["pv"])
                            for k in range(nk):
                                nc.tensor.matmul(pv[:], v_sb[:, k * 128:(k + 1) * 128],
                                                 pt_sb[:, k * 512:(k + 1) * 512],
                                                 start=(k == 0), stop=(k == nk - 1))
                            nc.scalar.copy(attnT[:, h * 512:(h + 1) * 512], pv[:])
                        # O projection for the strip
                        for ci in range(4):
                            c = 4 * t + ci
                            osb = pa.tile([128, HID], F32, tag="osb", bufs=bz["osb"])
                            for n in range(HID // 512):
                                op = psa.tile([128, 512], F32, tag="op", bufs=bz["op"])
                                for h in range(NH):
                                    nc.tensor.matmul(op[:], attnT[:, h * 512 + ci * 128: h * 512 + (ci + 1) * 128],
                                                     wo_sb[:, h * HID + n * 512: h * HID + (n + 1) * 512],
                                                     start=(h == 0), stop=(h == NH - 1))
                                nc.scalar.copy(osb[:, n * 512:(n + 1) * 512], op[:])
                            nc.sync.dma_start(out=out_d[c * 128:(c + 1) * 128, :], in_=osb[:])
                _rep.close()
    nc.compile()
    return nc


def _chunk_major(a, rows=128):
    """[R, C] -> [128, (R//128)*C] with row-chunk-major free layout."""
    r, c = a.shape
    return np.ascontiguousarray(a.reshape(r // rows, rows, c).transpose(1, 0, 2).reshape(rows, (r // rows) * c))


def make_in_map(x_b, cos, sin, wq_g, wk_g, wv_g, wo_g, S, HID):
    SC = S // 128
    xt = _chunk_major(np.ascontiguousarray(x_b.T)).astype(np.float16)
    wqkv = _chunk_major(np.concatenate([wq_g, wk_g, wv_g], axis=1)).astype(np.float16)
    cosr = cos[:S].reshape(SC, 128, D)
    cos4 = np.repeat(cosr[:, :, None, :], NH, axis=2).transpose(1, 0, 2, 3).reshape(128, SC * NH * D)
    sing = np.concatenate([-sin[:S, :64], sin[:S, 64:]], axis=1).reshape(SC, 128, D)
    sin4 = np.repeat(sing[:, :, None, :], NH, axis=2).transpose(1, 0, 2, 3).reshape(128, SC * NH * D)
    wo = _chunk_major(wo_g).astype(np.float16)
    return {
        "xt": xt,
        "wqkv": wqkv,
        "cos4": np.ascontiguousarray(cos4).astype(np.float16),
        "sin4": np.ascontiguousarray(sin4).astype(np.float16),
        "wo": wo,
    }


_NC_CACHE = {}


def _get_nc(S, HID):
    key = (S, HID)
    if key not in _NC_CACHE:
        _NC_CACHE[key] = build(S, HID)
    return _NC_CACHE[key]


def kernel(x, cos, sin, Wq, Wk, Wv, Wo):
    x = np.asarray(x, dtype=np.float32)
    cos = np.asarray(cos, dtype=np.float32)
    sin = np.asarray(sin, dtype=np.float32)
    Wq = np.asarray(Wq, dtype=np.float32)
    Wk = np.asarray(Wk, dtype=np.float32)
    Wv = np.asarray(Wv, dtype=np.float32)
    Wo = np.asarray(Wo, dtype=np.float32)
    B, S, HID = x.shape

    in_maps = []
    for i in range(8):
        b, g = i // 4, i % 4
        in_maps.append(make_in_map(
            x[b], cos, sin,
            Wq[:, g * NH * D:(g + 1) * NH * D],
            Wk[:, g * D:(g + 1) * D],
            Wv[:, g * D:(g + 1) * D],
            Wo[g * NH * D:(g + 1) * NH * D, :],
            S, HID))

    nc = _get_nc(S, HID)
    last_err = None
    for _attempt in range(3):
        try:
            res = run_bass_kernel_spmd(nc, in_maps, core_ids=list(range(8)), trace=False)
            break
        except Exception as e:  # flaky NRT_EXEC_UNIT_UNRECOVERABLE seen on first runs
            last_err = e
    else:
        raise last_err
    out = np.zeros((B, S, HID), dtype=np.float32)
    for i in range(8):
        b = i // 4
        out[b] += res.results[i]["out"]
    return out


# revision 16
# speedup vs baseline: 45.1675x; 6.6346x over previous
"""Trainium2 Bass kernel for GQA attention (B=2, S=2048, HID=2048, H=16, HKV=4, RoPE, causal).

Sharding: TP=4 over GQA groups (4 Q heads + 1 KV head per core) x DP=2 over batch.
Core i -> (batch = i // 4, group = i % 4). Each core computes a partial output
x @ Wo_shard for its head group; host sums the 4 partials per batch.

Per-core pipeline (all matmul operands fp16, fp32 PSUM accumulation):
  Phase P: qkv projections (xT chunks stationary, packed [Wq|Wk|Wv] moving),
           RoPE on DVE in [s, d] layout (free-axis half swap, host-signed sin),
           PE-transpose q/k head blocks -> qT/kT [d, s] fp16; v stays [s, d].
  Phase A: per 512-row query strip x head: scores = qT.T @ kT strips (N=512),
           causal mask via 4 precomputed diag-box masks (DVE add),
           exp on ACT (scale=1/sqrt(d), bias=-4) with fused row-sum accum,
           normalize P (tensor_scalar by reciprocal row-sum) -> fp16,
           PE-transpose P blocks -> PT [sk, sq], PV with v chunks stationary,
           then O-projection using PV's [d, sq] output directly as stationary.
"""
import sys
sys.path.insert(0, "/opt/trn_rl_repo")
import math
import numpy as np
import concourse.bass as bass
import concourse.mybir as mybir
import concourse.tile as tile
from concourse import bacc
from concourse.bass_utils import run_bass_kernel_spmd
from concourse.masks import make_identity

F16 = mybir.dt.float16
F32 = mybir.dt.float32
AF = mybir.ActivationFunctionType
ALU = mybir.AluOpType

NH = 4          # q heads per core
D = 128         # head dim
MASK_VAL = -1e9
EXP_BIAS = -4.0


DEFAULT_BUFS = dict(sc=2, tp=2, pv=2, op=2, prow=2, pn=2, pt=2, attnT=2,
                    qps=2, kvps=2, trps=2, osb=2)


def build(S=2048, HID=2048, repeat=1, bufs=None):
    bz = dict(DEFAULT_BUFS)
    if bufs:
        bz.update(bufs)
    SC = S // 128        # seq chunks
    NT = S // 512        # 512-wide query strips
    HC = HID // 128      # hidden (contraction) chunks
    QW = NH * D          # 512: q width per core
    scale = 1.0 / math.sqrt(D)

    nc = bacc.Bacc(None, target_bir_lowering=False, debug=False)
    with tile.TileContext(nc) as tc:
        with tc.tile_pool(name="dram", bufs=1, space="DRAM") as dram:
            xt_d = dram.tile([128, HC * S], F16, kind="ExternalInput", name="xt", uniquify=False)
            wqkv_d = dram.tile([128, HC * (QW + 256)], F16, kind="ExternalInput", name="wqkv", uniquify=False)
            cos_d = dram.tile([128, SC * QW], F16, kind="ExternalInput", name="cos4", uniquify=False)
            sin_d = dram.tile([128, SC * QW], F16, kind="ExternalInput", name="sin4", uniquify=False)
            wo_d = dram.tile([128, NH * HID], F16, kind="ExternalInput", name="wo", uniquify=False)
            out_d = dram.tile([S, HID], F32, kind="ExternalOutput", name="out", uniquify=False)

            # ---- persistent sbuf ----
            with tc.tile_pool(name="keep", bufs=1) as keep:
                qT_sb = keep.tile([128, NH * S], F16)   # [d, h*S + sq]
                kT_sb = keep.tile([128, S], F16)        # [d, sk]
                v_sb = keep.tile([128, S], F16)         # [sk%128, chunk*128 + d]
                ident = keep.tile([128, 128], F16)
                make_identity(nc, ident[:])
                ebias = keep.tile([128, 1], F32)
                nc.gpsimd.memset(ebias[:], EXP_BIAS)
                # triangular causal mask for the diagonal 128x128 block:
                # visible (0) iff key_pos <= query_pos, else MASK_VAL
                cmask = keep.tile([128, 128], F32)
                nc.gpsimd.memset(cmask[:], 0.0)
                nc.gpsimd.affine_select(
                    out=cmask[:], in_=cmask[:], compare_op=ALU.is_ge,
                    fill=MASK_VAL, base=0, pattern=[[-1, 128]], channel_multiplier=1,
                )

                # optional whole-body repeat loop (for HW timing builds)
                from contextlib import ExitStack
                _rep = ExitStack()
                if repeat > 1:
                    _rep.enter_context(tc.For_i(0, repeat, 1))

                # ---- phase P: projections + rope + transposes ----
                with tc.tile_pool(name="pp", bufs=1) as pp, \
                     tc.tile_pool(name="psp", bufs=2, space="PSUM") as psp:
                    xt_sb = pp.tile([128, HC * S], F16)
                    wqkv_sb = pp.tile([128, HC * (QW + 256)], F16)
                    cos_sb = pp.tile([128, SC * QW], F16)
                    sin_sb = pp.tile([128, SC * QW], F16)
                    for hh in range(HC):
                        nc.sync.dma_start(out=wqkv_sb[:, hh * (QW + 256):(hh + 1) * (QW + 256)],
                                          in_=wqkv_d[:, hh * (QW + 256):(hh + 1) * (QW + 256)])
                        nc.sync.dma_start(out=xt_sb[:, hh * S:(hh + 1) * S], in_=xt_d[:, hh * S:(hh + 1) * S])
                    nc.sync.dma_start(out=cos_sb[:], in_=cos_d[:])
                    nc.sync.dma_start(out=sin_sb[:], in_=sin_d[:])

                    for c in range(SC):
                        q_ps = psp.tile([128, QW], F32, tag="qps", bufs=bz["qps"])
                        kv_ps = psp.tile([128, 256], F32, tag="kvps", bufs=bz["kvps"])
                        for hh in range(HC):
                            xk = xt_sb[:, hh * S + c * 128: hh * S + (c + 1) * 128]
                            nc.tensor.matmul(q_ps[:], xk, wqkv_sb[:, hh * (QW + 256): hh * (QW + 256) + QW],
                                             start=(hh == 0), stop=(hh == HC - 1))
                            nc.tensor.matmul(kv_ps[:], xk, wqkv_sb[:, hh * (QW + 256) + QW: (hh + 1) * (QW + 256)],
                                             start=(hh == 0), stop=(hh == HC - 1))
                        # --- RoPE on q (4 heads batched) ---
                        q4 = q_ps[:].rearrange("p (h d) -> p h d", h=NH)
                        cos4v = cos_sb[:, c * QW:(c + 1) * QW].rearrange("p (h d) -> p h d", h=NH)
                        sin4v = sin_sb[:, c * QW:(c + 1) * QW].rearrange("p (h d) -> p h d", h=NH)
                        rot = pp.tile([128, QW], F32, tag="rot")
                        rot4 = rot[:].rearrange("p (h d) -> p h d", h=NH)
                        nc.vector.tensor_mul(rot4[:, :, 0:64], q4[:, :, 64:128], sin4v[:, :, 0:64])
                        nc.vector.tensor_mul(rot4[:, :, 64:128], q4[:, :, 0:64], sin4v[:, :, 64:128])
                        qc = pp.tile([128, QW], F32, tag="qc")
                        nc.vector.tensor_mul(qc[:], q_ps[:], cos_sb[:, c * QW:(c + 1) * QW])
                        q16 = pp.tile([128, QW], F16, tag="q16")
                        nc.vector.tensor_add(q16[:], qc[:], rot[:])
                        # --- RoPE on k (head 0 slices of cos/sin) ---
                        k1 = kv_ps[:, 0:128]
                        cos1 = cos_sb[:, c * QW: c * QW + 128]
                        sin1 = sin_sb[:, c * QW: c * QW + 128]
                        krot = pp.tile([128, 128], F32, tag="krot")
                        nc.vector.tensor_mul(krot[:, 0:64], k1[:, 64:128], sin1[:, 0:64])
                        nc.vector.tensor_mul(krot[:, 64:128], k1[:, 0:64], sin1[:, 64:128])
                        kc = pp.tile([128, 128], F32, tag="kc")
                        nc.vector.tensor_mul(kc[:], k1, cos1)
                        k16 = pp.tile([128, 128], F16, tag="k16")
                        nc.vector.tensor_add(k16[:], kc[:], krot[:])
                        # --- v to persistent [s, d] fp16 ---
                        nc.vector.tensor_copy(v_sb[:, c * 128:(c + 1) * 128], kv_ps[:, 128:256])
                        # --- transpose q heads + k into qT/kT ---
                        tr_ps = psp.tile([128, 640], F16, tag="trps", bufs=bz["trps"])
                        for h in range(NH):
                            nc.tensor.transpose(tr_ps[:, h * 128:(h + 1) * 128], q16[:, h * 128:(h + 1) * 128], ident[:])
                        nc.tensor.transpose(tr_ps[:, 512:640], k16[:], ident[:])
                        qT_view = qT_sb[:].rearrange("p (h s) -> p h s", h=NH)[:, :, c * 128:(c + 1) * 128]
                        nc.vector.tensor_copy(qT_view, tr_ps[:, 0:512].rearrange("p (h s) -> p h s", h=NH))
                        nc.vector.tensor_copy(kT_sb[:, c * 128:(c + 1) * 128], tr_ps[:, 512:640])

                # ---- phase A: attention + output projection ----
                with tc.tile_pool(name="pa", bufs=1) as pa, \
                     tc.tile_pool(name="psa", bufs=2, space="PSUM") as psa:
                    wo_sb = pa.tile([128, NH * HID], F16)
                    nc.sync.dma_start(out=wo_sb[:], in_=wo_d[:])
                    for t in range(NT):
                        nk = 4 * t + 4            # sk chunks in play for this strip
                        attnT = pa.tile([128, NH * 512], F16, tag="attnT", bufs=bz["attnT"])
                        for h in range(NH):
                            pt_sb = pa.tile([128, nk * 512], F16, tag="pt", bufs=bz["pt"])
                            prow = pa.tile([128, S], F16, tag="prow", bufs=bz["prow"])
                            rs = pa.tile([128, 4], F32, tag="rs")
                            for ci in range(4):
                                c = 4 * t + ci
                                nvis = (c + 1) * 128   # visible key width for this row block
                                qTc = qT_sb[:, h * S + c * 128: h * S + (c + 1) * 128]
                                for j in range(t + 1):
                                    w = 512 if j < t else (ci + 1) * 128
                                    sc = psa.tile([128, 512], F32, tag="sc", bufs=bz["sc"])
                                    nc.tensor.matmul(sc[:, 0:w], qTc, kT_sb[:, j * 512: j * 512 + w],
                                                     start=True, stop=True)
                                    if j == t:
                                        nc.vector.tensor_add(sc[:, ci * 128:(ci + 1) * 128],
                                                             sc[:, ci * 128:(ci + 1) * 128], cmask[:])
                                    nc.scalar.activation(prow[:, j * 512: j * 512 + w], sc[:, 0:w], AF.Exp,
                                                         scale=scale, bias=ebias[:],
                                                         accum_out=rs[:, j:j + 1])
                                rst = pa.tile([128, 1], F32, tag="rst")
                                if t == 0:
                                    nc.vector.tensor_copy(rst[:], rs[:, 0:1])
                                elif t == 1:
                                    nc.vector.tensor_add(rst[:], rs[:, 0:1], rs[:, 1:2])
                                else:
                                    nc.vector.tensor_add(rst[:], rs[:, 0:1], rs[:, 1:2])
                                    for j in range(2, t + 1):
                                        nc.vector.tensor_add(rst[:], rst[:], rs[:, j:j + 1])
                                rcp = pa.tile([128, 1], F32, tag="rcp")
                                nc.vector.reciprocal(rcp[:], rst[:])
                                pn = pa.tile([128, S], F16, tag="pn", bufs=bz["pn"])
                                nc.vector.tensor_scalar_mul(pn[:, 0:nvis], prow[:, 0:nvis], rcp[:])
                                # transpose the c+1 visible P blocks into PT strip layout
                                for k0 in range(0, c + 1, 4):
                                    kb = min(4, c + 1 - k0)
                                    tp = psa.tile([128, 512], F16, tag="tp", bufs=bz["tp"])
                                    for kk in range(kb):
                                        nc.tensor.transpose(tp[:, kk * 128:(kk + 1) * 128],
                                                            pn[:, (k0 + kk) * 128:(k0 + kk + 1) * 128], ident[:])
                                    ptv = pt_sb[:].rearrange("p (k s) -> p k s", s=512)[:, k0:k0 + kb, ci * 128:(ci + 1) * 128]
                                    nc.vector.tensor_copy(ptv, tp[:, 0:kb * 128].rearrange("p (k s) -> p k s", s=128))
                                # blocks k > c are all-zero P: memset their PT slots (Pool engine)
                                if c + 1 < nk:
                                    zv = pt_sb[:].rearrange("p (k s) -> p k s", s=512)[:, c + 1:nk, ci * 128:(ci + 1) * 128]
                                    nc.gpsimd.memset(zv, 0.0)
                            # PV for this (h, t)
                            pv = psa.tile([128, 512], F32, tag="pv", bufs=bz["pv"])
                            for k in range(nk):
                                nc.tensor.matmul(pv[:], v_sb[:, k * 128:(k + 1) * 128],
                                                 pt_sb[:, k * 512:(k + 1) * 512],
                                                 start=(k == 0), stop=(k == nk - 1))
                            nc.scalar.copy(attnT[:, h * 512:(h + 1) * 512], pv[:])
                        # O projection for the strip
                        for ci in range(4):
                            c = 4 * t + ci
                            osb = pa.tile([128, HID], F32, tag="osb", bufs=bz["osb"])
                            for n in range(HID // 512):
                                op = psa.tile([128, 512], F32, tag="op", bufs=bz["op"])
                                for h in range(NH):
                                    nc.tensor.matmul(op[:], attnT[:, h * 512 + ci * 128: h * 512 + (ci + 1) * 128],
                                                     wo_sb[:, h * HID + n * 512: h * HID + (n + 1) * 512],
                                                     start=(h == 0), stop=(h == NH - 1))
                                nc.scalar.copy(osb[:, n * 512:(n + 1) * 512], op[:])
                            nc.sync.dma_start(out=out_d[c * 128:(c + 1) * 128, :], in_=osb[:])
                _rep.close()
    nc.compile()
    return nc


def _chunk_major(a, rows=128):
    """[R, C] -> [128, (R//128)*C] with row-chunk-major free layout."""
    r, c = a.shape
    return np.ascontiguousarray(a.reshape(r // rows, rows, c).transpose(1, 0, 2).reshape(rows, (r // rows) * c))


def make_in_map(x_b, cos, sin, wq_g, wk_g, wv_g, wo_g, S, HID):
    SC = S // 128
    xt = _chunk_major(np.ascontiguousarray(x_b.T)).astype(np.float16)
    wqkv = _chunk_major(np.concatenate([wq_g, wk_g, wv_g], axis=1)).astype(np.float16)
    cosr = cos[:S].reshape(SC, 128, D)
    cos4 = np.repeat(cosr[:, :, None, :], NH, axis=2).transpose(1, 0, 2, 3).reshape(128, SC * NH * D)
    sing = np.concatenate([-sin[:S, :64], sin[:S, 64:]], axis=1).reshape(SC, 128, D)
    sin4 = np.repeat(sing[:, :, None, :], NH, axis=2).transpose(1, 0, 2, 3).reshape(128, SC * NH * D)
    wo = _chunk_major(wo_g).astype(np.float16)
    return {
        "xt": xt,
        "wqkv": wqkv,
        "cos4": np.ascontiguousarray(cos4).astype(np.float16),
        "sin4": np.ascontiguousarray(sin4).astype(np.float16),
        "wo": wo,
    }


_NC_CACHE = {}


def _get_nc(S, HID):
    key = (S, HID)
    if key not in _NC_CACHE:
        _NC_CACHE[key] = build(S, HID)
    return _NC_CACHE[key]


def kernel(x, cos, sin, Wq, Wk, Wv, Wo):
    x = np.asarray(x, dtype=np.float32)
    cos = np.asarray(cos, dtype=np.float32)
    sin = np.asarray(sin, dtype=np.float32)
    Wq = np.asarray(Wq, dtype=np.float32)
    Wk = np.asarray(Wk, dtype=np.float32)
    Wv = np.asarray(Wv, dtype=np.float32)
    Wo = np.asarray(Wo, dtype=np.float32)
    B, S, HID = x.shape

    in_maps = []
    for i in range(8):
        b, g = i // 4, i % 4
        in_maps.append(make_in_map(
            x[b], cos, sin,
            Wq[:, g * NH * D:(g + 1) * NH * D],
            Wk[:, g * D:(g + 1) * D],
            Wv[:, g * D:(g + 1) * D],
            Wo[g * NH * D:(g + 1) * NH * D, :],
            S, HID))

    nc = _get_nc(S, HID)
    last_err = None
    for _attempt in range(3):
        try:
            res = run_bass_kernel_spmd(nc, in_maps, core_ids=list(range(8)), trace=False)
            break
        except Exception as e:  # flaky NRT_EXEC_UNIT_UNRECOVERABLE seen on first runs
            last_err = e
    else:
        raise last_err
    out = np.zeros((B, S, HID), dtype=np.float32)
    for i in range(8):
        b = i // 4
        out[b] += res.results[i]["out"]
    return out


# revision 25
# speedup vs baseline: 56.5836x; 1.2528x over previous
"""Trainium2 Bass kernel for GQA attention (B=2, S=2048, HID=2048, H=16, HKV=4, RoPE, causal).

Sharding: TP=4 over GQA groups (4 Q heads + 1 KV head per core) x DP=2 over batch.
Core i -> (batch = i // 4, group = i % 4). Each core computes a partial output
x @ Wo_shard for its head group; host sums the 4 partials per batch.

Per-core pipeline (all matmul operands fp16, fp32 PSUM accumulation):
  Phase P: qkv projections (xT chunks stationary, packed [Wq|Wk|Wv] moving),
           RoPE on DVE in [s, d] layout (free-axis half swap, host-signed sin),
           PE-transpose q/k head blocks -> qT/kT [d, s] fp16; v stays [s, d].
  Phase A: per 512-row query strip x head: scores = qT.T @ kT strips (N=512),
           causal mask via 4 precomputed diag-box masks (DVE add),
           exp on ACT (scale=1/sqrt(d), bias=-4) with fused row-sum accum,
           normalize P (tensor_scalar by reciprocal row-sum) -> fp16,
           PE-transpose P blocks -> PT [sk, sq], PV with v chunks stationary,
           then O-projection using PV's [d, sq] output directly as stationary.
"""
import sys
sys.path.insert(0, "/opt/trn_rl_repo")
import math
import numpy as np
import concourse.mybir as mybir
import concourse.tile as tile
from concourse import bacc
from concourse.bass_utils import run_bass_kernel_spmd
from concourse.masks import make_identity

F16 = mybir.dt.float16
F32 = mybir.dt.float32
AF = mybir.ActivationFunctionType
ALU = mybir.AluOpType

NH = 4          # q heads per core
D = 128         # head dim
MASK_VAL = -1e9
EXP_BIAS = -4.0


DEFAULT_BUFS = dict(sc=2, tp=2, pv=2, op=2, prow=2, pn=2, pt=2, attnT=2,
                    qps=2, kvps=2, trps=2, osb=2)


def build(S=2048, HID=2048, repeat=1, bufs=None, dmat=False, split_norm=False):
    bz = dict(DEFAULT_BUFS)
    if bufs:
        bz.update(bufs)
    SC = S // 128        # seq chunks
    NT = S // 512        # 512-wide query strips
    HC = HID // 128      # hidden (contraction) chunks
    QW = NH * D          # 512: q width per core
    scale = 1.0 / math.sqrt(D)

    nc = bacc.Bacc(None, target_bir_lowering=False, debug=False)
    with tile.TileContext(nc) as tc:
        with tc.tile_pool(name="dram", bufs=1, space="DRAM") as dram:
            xt_d = dram.tile([128, HC * S], F16, kind="ExternalInput", name="xt", uniquify=False)
            wqkv_d = dram.tile([128, HC * (QW + 256)], F16, kind="ExternalInput", name="wqkv", uniquify=False)
            cos_d = dram.tile([128, SC * QW], F16, kind="ExternalInput", name="cos4", uniquify=False)
            sin_d = dram.tile([128, SC * QW], F16, kind="ExternalInput", name="sin4", uniquify=False)
            wo_d = dram.tile([128, NH * HID], F16, kind="ExternalInput", name="wo", uniquify=False)
            out_d = dram.tile([S, HID], F32, kind="ExternalOutput", name="out", uniquify=False)

            # ---- persistent sbuf ----
            with tc.tile_pool(name="keep", bufs=1) as keep:
                qT_sb = keep.tile([128, NH * S], F16)   # [d, h*S + sq]
                kT_sb = keep.tile([128, S], F16)        # [d, sk]
                v_sb = keep.tile([128, S], F16)         # [sk%128, chunk*128 + d]
                ident = keep.tile([128, 128], F16)
                make_identity(nc, ident[:])
                ebias = keep.tile([128, 1], F32)
                nc.gpsimd.memset(ebias[:], EXP_BIAS)
                # triangular causal mask for the diagonal 128x128 block:
                # visible (0) iff key_pos <= query_pos, else MASK_VAL
                cmask = keep.tile([128, 128], F32)
                nc.gpsimd.memset(cmask[:], 0.0)
                nc.gpsimd.affine_select(
                    out=cmask[:], in_=cmask[:], compare_op=ALU.is_ge,
                    fill=MASK_VAL, base=0, pattern=[[-1, 128]], channel_multiplier=1,
                )

                # optional whole-body repeat loop (for HW timing builds)
                from contextlib import ExitStack
                _rep = ExitStack()
                if repeat > 1:
                    _rep.enter_context(tc.For_i(0, repeat, 1))

                # ---- phase P: projections + rope + transposes ----
                with tc.tile_pool(name="pp", bufs=1) as pp, \
                     tc.tile_pool(name="psp", bufs=2, space="PSUM") as psp:
                    xt_sb = pp.tile([128, HC * S], F16)
                    wqkv_sb = pp.tile([128, HC * (QW + 256)], F16)
                    cos_sb = pp.tile([128, SC * QW], F16)
                    sin_sb = pp.tile([128, SC * QW], F16)
                    for hh in range(HC):
                        nc.sync.dma_start(out=wqkv_sb[:, hh * (QW + 256):(hh + 1) * (QW + 256)],
                                          in_=wqkv_d[:, hh * (QW + 256):(hh + 1) * (QW + 256)])
                        nc.sync.dma_start(out=xt_sb[:, hh * S:(hh + 1) * S], in_=xt_d[:, hh * S:(hh + 1) * S])
                    nc.sync.dma_start(out=cos_sb[:], in_=cos_d[:])
                    nc.sync.dma_start(out=sin_sb[:], in_=sin_d[:])

                    for c in range(SC):
                        q_ps = psp.tile([128, QW], F32, tag="qps", bufs=bz["qps"])
                        kv_ps = psp.tile([128, 256], F32, tag="kvps", bufs=bz["kvps"])
                        for hh in range(HC):
                            xk = xt_sb[:, hh * S + c * 128: hh * S + (c + 1) * 128]
                            nc.tensor.matmul(q_ps[:], xk, wqkv_sb[:, hh * (QW + 256): hh * (QW + 256) + QW],
                                             start=(hh == 0), stop=(hh == HC - 1))
                            nc.tensor.matmul(kv_ps[:], xk, wqkv_sb[:, hh * (QW + 256) + QW: (hh + 1) * (QW + 256)],
                                             start=(hh == 0), stop=(hh == HC - 1))
                        # --- RoPE on q (4 heads batched) ---
                        q4 = q_ps[:].rearrange("p (h d) -> p h d", h=NH)
                        cos4v = cos_sb[:, c * QW:(c + 1) * QW].rearrange("p (h d) -> p h d", h=NH)
                        sin4v = sin_sb[:, c * QW:(c + 1) * QW].rearrange("p (h d) -> p h d", h=NH)
                        rot = pp.tile([128, QW], F32, tag="rot")
                        rot4 = rot[:].rearrange("p (h d) -> p h d", h=NH)
                        nc.vector.tensor_mul(rot4[:, :, 0:64], q4[:, :, 64:128], sin4v[:, :, 0:64])
                        nc.vector.tensor_mul(rot4[:, :, 64:128], q4[:, :, 0:64], sin4v[:, :, 64:128])
                        qc = pp.tile([128, QW], F32, tag="qc")
                        nc.vector.tensor_mul(qc[:], q_ps[:], cos_sb[:, c * QW:(c + 1) * QW])
                        q16 = pp.tile([128, QW], F16, tag="q16")
                        nc.vector.tensor_add(q16[:], qc[:], rot[:])
                        # --- RoPE on k (head 0 slices of cos/sin) ---
                        k1 = kv_ps[:, 0:128]
                        cos1 = cos_sb[:, c * QW: c * QW + 128]
                        sin1 = sin_sb[:, c * QW: c * QW + 128]
                        krot = pp.tile([128, 128], F32, tag="krot")
                        nc.vector.tensor_mul(krot[:, 0:64], k1[:, 64:128], sin1[:, 0:64])
                        nc.vector.tensor_mul(krot[:, 64:128], k1[:, 0:64], sin1[:, 64:128])
                        kc = pp.tile([128, 128], F32, tag="kc")
                        nc.vector.tensor_mul(kc[:], k1, cos1)
                        k16 = pp.tile([128, 128], F16, tag="k16")
                        nc.vector.tensor_add(k16[:], kc[:], krot[:])
                        # --- v to persistent [s, d] fp16 ---
                        nc.vector.tensor_copy(v_sb[:, c * 128:(c + 1) * 128], kv_ps[:, 128:256])
                        # --- transpose q heads + k into qT/kT ---
                        tr_ps = psp.tile([128, 640], F16, tag="trps", bufs=bz["trps"])
                        for h in range(NH):
                            nc.tensor.transpose(tr_ps[:, h * 128:(h + 1) * 128], q16[:, h * 128:(h + 1) * 128], ident[:])
                        nc.tensor.transpose(tr_ps[:, 512:640], k16[:], ident[:])
                        qT_view = qT_sb[:].rearrange("p (h s) -> p h s", h=NH)[:, :, c * 128:(c + 1) * 128]
                        nc.vector.tensor_copy(qT_view, tr_ps[:, 0:512].rearrange("p (h s) -> p h s", h=NH))
                        nc.vector.tensor_copy(kT_sb[:, c * 128:(c + 1) * 128], tr_ps[:, 512:640])

                # ---- phase A: attention + output projection ----
                with tc.tile_pool(name="pa", bufs=1) as pa, \
                     tc.tile_pool(name="psa", bufs=2, space="PSUM") as psa:
                    wo_sb = pa.tile([128, NH * HID], F16)
                    nc.sync.dma_start(out=wo_sb[:], in_=wo_d[:])
                    for t in range(NT):
                        nk = 4 * t + 4            # sk chunks in play for this strip
                        attnT = pa.tile([128, NH * 512], F16, tag="attnT", bufs=bz["attnT"])
                        for h in range(NH):
                            pt_sb = pa.tile([128, nk * 512], F16, tag="pt", bufs=bz["pt"])
                            prow = pa.tile([128, S], F16, tag="prow", bufs=bz["prow"])
                            rs = pa.tile([128, 4], F32, tag="rs")
                            for ci in range(4):
                                c = 4 * t + ci
                                nvis = (c + 1) * 128   # visible key width for this row block
                                qTc = qT_sb[:, h * S + c * 128: h * S + (c + 1) * 128]
                                for j in range(t + 1):
                                    w = 512 if j < t else (ci + 1) * 128
                                    sc = psa.tile([128, 512], F32, tag="sc", bufs=bz["sc"])
                                    nc.tensor.matmul(sc[:, 0:w], qTc, kT_sb[:, j * 512: j * 512 + w],
                                                     start=True, stop=True)
                                    if j == t:
                                        nc.vector.tensor_add(sc[:, ci * 128:(ci + 1) * 128],
                                                             sc[:, ci * 128:(ci + 1) * 128], cmask[:])
                                    nc.scalar.activation(prow[:, j * 512: j * 512 + w], sc[:, 0:w], AF.Exp,
                                                         scale=scale, bias=ebias[:],
                                                         accum_out=rs[:, j:j + 1])
                                rst = pa.tile([128, 1], F32, tag="rst")
                                if t == 0:
                                    nc.vector.tensor_copy(rst[:], rs[:, 0:1])
                                elif t == 1:
                                    nc.vector.tensor_add(rst[:], rs[:, 0:1], rs[:, 1:2])
                                else:
                                    nc.vector.tensor_add(rst[:], rs[:, 0:1], rs[:, 1:2])
                                    for j in range(2, t + 1):
                                        nc.vector.tensor_add(rst[:], rst[:], rs[:, j:j + 1])
                                rcp = pa.tile([128, 1], F32, tag="rcp")
                                nc.vector.reciprocal(rcp[:], rst[:])
                                pn = pa.tile([128, S], F16, tag="pn", bufs=bz["pn"])
                                if split_norm:
                                    for z0 in range(0, nvis, 512):
                                        zw = min(512, nvis - z0)
                                        nc.vector.tensor_scalar_mul(pn[:, z0:z0 + zw], prow[:, z0:z0 + zw], rcp[:])
                                else:
                                    nc.vector.tensor_scalar_mul(pn[:, 0:nvis], prow[:, 0:nvis], rcp[:])
                                # transpose the c+1 visible P blocks into PT strip layout
                                if dmat:
                                    for k in range(c + 1):
                                        nc.sync.dma_start_transpose(
                                            pt_sb[:, k * 512 + ci * 128: k * 512 + (ci + 1) * 128],
                                            pn[:, k * 128:(k + 1) * 128])
                                else:
                                    for k0 in range(0, c + 1, 4):
                                        kb = min(4, c + 1 - k0)
                                        tp = psa.tile([128, 512], F16, tag="tp", bufs=bz["tp"])
                                        for kk in range(kb):
                                            nc.tensor.transpose(tp[:, kk * 128:(kk + 1) * 128],
                                                                pn[:, (k0 + kk) * 128:(k0 + kk + 1) * 128], ident[:])
                                        ptv = pt_sb[:].rearrange("p (k s) -> p k s", s=512)[:, k0:k0 + kb, ci * 128:(ci + 1) * 128]
                                        nc.vector.tensor_copy(ptv, tp[:, 0:kb * 128].rearrange("p (k s) -> p k s", s=128))
                                # blocks k > c are all-zero P: memset their PT slots (Pool engine)
                                if c + 1 < nk:
                                    zv = pt_sb[:].rearrange("p (k s) -> p k s", s=512)[:, c + 1:nk, ci * 128:(ci + 1) * 128]
                                    nc.gpsimd.memset(zv, 0.0)
                            # PV for this (h, t)
                            pv = psa.tile([128, 512], F32, tag="pv", bufs=bz["pv"])
                            for k in range(nk):
                                nc.tensor.matmul(pv[:], v_sb[:, k * 128:(k + 1) * 128],
                                                 pt_sb[:, k * 512:(k + 1) * 512],
                                                 start=(k == 0), stop=(k == nk - 1))
                            nc.scalar.copy(attnT[:, h * 512:(h + 1) * 512], pv[:])
                        # O projection for the strip
                        for ci in range(4):
                            c = 4 * t + ci
                            osb = pa.tile([128, HID], F32, tag="osb", bufs=bz["osb"])
                            for n in range(HID // 512):
                                op = psa.tile([128, 512], F32, tag="op", bufs=bz["op"])
                                for h in range(NH):
                                    nc.tensor.matmul(op[:], attnT[:, h * 512 + ci * 128: h * 512 + (ci + 1) * 128],
                                                     wo_sb[:, h * HID + n * 512: h * HID + (n + 1) * 512],
                                                     start=(h == 0), stop=(h == NH - 1))
                                nc.scalar.copy(osb[:, n * 512:(n + 1) * 512], op[:])
                            nc.sync.dma_start(out=out_d[c * 128:(c + 1) * 128, :], in_=osb[:])
                _rep.close()
    nc.compile()
    return nc


def _chunk_major(a, rows=128):
    """[R, C] -> [128, (R//128)*C] with row-chunk-major free layout."""
    r, c = a.shape
    return np.ascontiguousarray(a.reshape(r // rows, rows, c).transpose(1, 0, 2).reshape(rows, (r // rows) * c))


def make_in_map(x_b, cos, sin, wq_g, wk_g, wv_g, wo_g, S, HID):
    SC = S // 128
    xt = _chunk_major(np.ascontiguousarray(x_b.T)).astype(np.float16)
    wqkv = _chunk_major(np.concatenate([wq_g, wk_g, wv_g], axis=1)).astype(np.float16)
    cosr = cos[:S].reshape(SC, 128, D)
    cos4 = np.repeat(cosr[:, :, None, :], NH, axis=2).transpose(1, 0, 2, 3).reshape(128, SC * NH * D)
    sing = np.concatenate([-sin[:S, :64], sin[:S, 64:]], axis=1).reshape(SC, 128, D)
    sin4 = np.repeat(sing[:, :, None, :], NH, axis=2).transpose(1, 0, 2, 3).reshape(128, SC * NH * D)
    wo = _chunk_major(wo_g).astype(np.float16)
    return {
        "xt": xt,
        "wqkv": wqkv,
        "cos4": np.ascontiguousarray(cos4).astype(np.float16),
        "sin4": np.ascontiguousarray(sin4).astype(np.float16),
        "wo": wo,
    }


_NC_CACHE = {}

# best-measured configuration (updated as experiments conclude)
BEST_BUFS = dict(sc=3, pv=1, prow=3, pn=3, qps=3, kvps=3)
BEST_DMAT = False
BEST_SPLIT_NORM = False


def _get_nc(S, HID):
    key = (S, HID)
    if key not in _NC_CACHE:
        _NC_CACHE[key] = build(S, HID, bufs=BEST_BUFS, dmat=BEST_DMAT,
                               split_norm=BEST_SPLIT_NORM)
    return _NC_CACHE[key]


def kernel(x, cos, sin, Wq, Wk, Wv, Wo):
    x = np.asarray(x, dtype=np.float32)
    cos = np.asarray(cos, dtype=np.float32)
    sin = np.asarray(sin, dtype=np.float32)
    Wq = np.asarray(Wq, dtype=np.float32)
    Wk = np.asarray(Wk, dtype=np.float32)
    Wv = np.asarray(Wv, dtype=np.float32)
    Wo = np.asarray(Wo, dtype=np.float32)
    B, S, HID = x.shape

    in_maps = []
    for i in range(8):
        b, g = i // 4, i % 4
        in_maps.append(make_in_map(
            x[b], cos, sin,
            Wq[:, g * NH * D:(g + 1) * NH * D],
            Wk[:, g * D:(g + 1) * D],
            Wv[:, g * D:(g + 1) * D],
            Wo[g * NH * D:(g + 1) * NH * D, :],
            S, HID))

    nc = _get_nc(S, HID)
    last_err = None
    for _attempt in range(3):
        try:
            res = run_bass_kernel_spmd(nc, in_maps, core_ids=list(range(8)), trace=False)
            break
        except Exception as e:  # flaky NRT_EXEC_UNIT_UNRECOVERABLE seen on first runs
            last_err = e
            import time as _time
            _time.sleep(5.0)
    else:
        raise last_err
    out = np.zeros((B, S, HID), dtype=np.float32)
    for i in range(8):
        b = i // 4
        out[b] += res.results[i]["out"]
    return out
